# revision 45
# baseline (speedup 1.0000x reference)
"""Trainium2 kernel for nn_DistanceField.

Math: for each batch b, pairwise distances d_ij over N=256 particles feed a
small MLP f(d) (scalar), and the output field is
    v_i = (1/(N-1)) sum_{j!=i} (x_i - x_j) f(d_ij)
        = (x_i * S_i - (F @ x)_i) / (N-1),   S_i = sum_j F_ij,  F_ij = f(d_ij)
    y   = v - mean_i(v)
    trace_b = (1/N) * sum_{i,j!=i} g(d_ij),  g = 3 f + (d - EPS/d) f'(d)

f and g are smooth 1-D functions of d only (t and the MLP weights are shared
by every pair), so the device evaluates them via a tanh-feature fit
    f(d) ~= sum_m cf_m tanh(sigma_m d + beta_m)   (same basis for g)
fitted host-side in float64 against the exact MLP at kernel() time (ACT Tanh
is hardware-exact for any argument; Sin is only valid on [-pi, pi]). The
device computes, per batch: pairwise d via a Gram matmul + Sqrt, the tanh
basis via a tiny PE outer-product (fp32r, 1 cycle/row) + one ACT Tanh pass,
f/g via PE matmuls with the coefficient matrix, then the F-matrix reductions
(row sums, F @ x, total g) via PE matmuls. F is symmetric, so only the
(I0, all-j) strip and the (I1, J1) block of each 256x256 pair block are
evaluated; the (I1, J0) quarter is recovered with one PE transpose.
Sharding: data-parallel over batch, 4 batches per core on 8 cores.
Everything stays on-chip; no HBM intermediates.
"""
import numpy as np

B, N, D, H = 32, 256, 3, 64
EPS = 1e-6
NCORES = 8
BPC = B // NCORES          # batches per core
S_STREAMS = 8              # pair streams per outer-product group
N_FEAT = 128 // S_STREAMS  # tanh features per stream

_NC = None                 # cached Bass module
_TRACE = False             # set True to capture an NTFF trace (test harness)
_LAST_RES = None           # BassKernelResults of the last run


# ------------------------------------------------------------------ host math
def _mlp_fg(d, t, W1, b1, W2, b2, W3, b3):
    """Exact float64 f(d), g(d) of the reference MLP."""
    d = np.asarray(d, np.float64)
    W1 = np.asarray(W1, np.float64); b1 = np.asarray(b1, np.float64)
    W2 = np.asarray(W2, np.float64); b2 = np.asarray(b2, np.float64)
    W3 = np.asarray(W3, np.float64); b3 = np.asarray(b3, np.float64)
    t = np.float64(np.asarray(t).reshape(()))
    c = t * W1[1] + b1
    h1 = np.tanh(d[:, None] * W1[0][None, :] + c)
    h2 = np.tanh(h1 @ W2 + b2)
    f = (h2 @ W3)[:, 0] + b3[0]
    u1 = (1.0 - h1 * h1) * W1[0][None, :]
    u2 = (1.0 - h2 * h2) * (u1 @ W2)
    fp = (u2 @ W3)[:, 0]
    g = D * f + (d - EPS / d) * fp
    return f, g


def _fit(t, W1, b1, W2, b2, W3, b3, dmin, dmax, n_feat, sig_mult=0.5):
    """Fit f and g on [dmin,dmax] in a tanh-sigmoid basis
        basis_m(d) = tanh(sigma_m * d + beta_m)
    (feature 0 is the DC term tanh(37)=1). ACT Tanh is exact for any
    argument, unlike Sin which is only valid on [-pi,pi]. A short
    Levenberg-Marquardt pass (variable projection: coefficients re-solved in
    closed form per step) tunes the per-feature scales and centers, which
    buys ~40x accuracy over a fixed uniform layout at 16 features.
    Returns sigma, beta, cf, cg as float32 arrays."""
    pad = 0.05 * (dmax - dmin)
    a = max(dmin - pad, 1e-4)
    b = dmax + pad
    G = 4096
    center, L = 0.5 * (a + b), (b - a)
    u = np.cos(np.pi * (np.arange(G) + 0.5) / G)
    grid = np.concatenate([center + 0.5 * L * u, np.linspace(a, b, G)])
    fg, gg = _mlp_fg(grid, t, W1, b1, W2, b2, W3, b3)
    wg = np.abs(fg).max() / max(np.abs(gg).max(), 1e-30)
    # ridge strong enough to keep coefficients O(0.1): large canceling
    # coefficient pairs amplify the ~2^-13 fp32r rounding of the sigma*d
    # products on device
    lam = 3e-7 * len(grid)
    eye = np.eye(n_feat)

    def solve(theta):
        sig = theta[:n_feat - 1]
        tc = theta[n_feat - 1:]
        sigma = np.concatenate([[0.0], sig])
        beta = np.concatenate([[37.0], -sig * tc])
        A = np.tanh(np.outer(grid, sigma) + beta[None, :])
        ATA = A.T @ A + lam * eye
        cf = np.linalg.solve(ATA, A.T @ fg)
        cg = np.linalg.solve(ATA, A.T @ gg)
        r = np.concatenate([A @ cf - fg, wg * (A @ cg - gg)])
        return r, sigma, beta, cf, cg

    tk0 = np.linspace(a, b, n_feat - 1)
    theta = np.concatenate([np.full(n_feat - 1, sig_mult / (tk0[1] - tk0[0])),
                            tk0])
    best = solve(theta)
    best_theta, best_sse = theta, float((best[0] ** 2).sum())
    try:
        mu = 1e-3
        for _ in range(20):
            r0, *_ = solve(theta)
            J = np.zeros((len(r0), len(theta)))
            for j in range(len(theta)):
                dth = theta.copy()
                h = 1e-5 * max(1.0, abs(theta[j]))
                dth[j] += h
                J[:, j] = (solve(dth)[0] - r0) / h
            JTJ = J.T @ J
            JTr = J.T @ r0
            for _ in range(8):
                step = np.linalg.solve(
                    JTJ + mu * np.diag(np.diag(JTJ) + 1e-12), JTr)
                thn = theta - step
                rn, *_ = solve(thn)
                sse = float((rn ** 2).sum())
                if sse < float((r0 ** 2).sum()):
                    theta = thn
                    mu = max(mu / 3, 1e-8)
                    if sse < best_sse:
                        best_theta, best_sse = thn, sse
                    break
                mu *= 10
            else:
                break
    except np.linalg.LinAlgError:
        pass  # fall back to the best (possibly initial uniform) layout
    _, sigma, beta, cf, cg = solve(best_theta)
    return (sigma.astype(np.float32), beta.astype(np.float32),
            cf.astype(np.float32), cg.astype(np.float32))


# --------------------------------------------------------------- bass builder
def _build_nc():
    import concourse.mybir as mybir
    from concourse import bacc
    from concourse.tile import TileContext

    dt = mybir.dt.float32
    dtr = mybir.dt.float32r
    Alu = mybir.AluOpType
    Act = mybir.ActivationFunctionType
    from concourse.tile import add_dep_helper
    S = S_STREAMS
    NG = 128 // S              # groups per batch (16)
    nc = bacc.Bacc("TRN2", target_bir_lowering=False, debug=False)

    xc_h = nc.declare_dram_parameter("xc", [3, BPC * N], dt, isOutput=False)
    m2xc_h = nc.declare_dram_parameter("m2xc", [3, BPC * N], dt, isOutput=False)
    nv_h = nc.declare_dram_parameter("nv", [1, BPC * N], dt, isOutput=False)
    xp1_h = nc.declare_dram_parameter("xp1", [128, BPC * 8], dt, isOutput=False)
    omt_h = nc.declare_dram_parameter("omt", [128, 128 * (64 // S)], dt,
                                      isOutput=False)
    phc_h = nc.declare_dram_parameter("phc", [128, 1], dt, isOutput=False)
    cmt_h = nc.declare_dram_parameter("cmt", [128, 2 * S], dt, isOutput=False)
    dmk_h = nc.declare_dram_parameter("dmk", [128, 128], dt, isOutput=False)
    outv_h = nc.declare_dram_parameter("outv", [BPC, N, 4], dt, isOutput=True)
    outt_h = nc.declare_dram_parameter("outt", [1, BPC], dt, isOutput=True)

    with TileContext(nc) as tc:
        from contextlib import ExitStack
        with ExitStack() as ctx:
            consts = ctx.enter_context(tc.tile_pool(name="consts", bufs=1))
            ddp = ctx.enter_context(tc.tile_pool(name="dd", bufs=1))
            d2sbp = ctx.enter_context(tc.tile_pool(name="d2sb", bufs=2))
            basp = ctx.enter_context(tc.tile_pool(name="bas", bufs=3))
            stgp = ctx.enter_context(tc.tile_pool(name="stg", bufs=2))
            ftp = ctx.enter_context(tc.tile_pool(name="ft", bufs=2))
            outp = ctx.enter_context(tc.tile_pool(name="outs", bufs=2))
            zpsum = ctx.enter_context(tc.tile_pool(name="zp", bufs=2, space="PSUM"))
            ppsum = ctx.enter_context(tc.tile_pool(name="pp", bufs=1, space="PSUM"))
            d2psum = ctx.enter_context(tc.tile_pool(name="d2p", bufs=1, space="PSUM"))
            smpsum = ctx.enter_context(tc.tile_pool(name="smp", bufs=2, space="PSUM"))

            # ---- load constants / inputs
            xc_sb = consts.tile([3, BPC * N], dt, tag="xc")
            nc.sync.dma_start(out=xc_sb[:], in_=xc_h[:])
            m2xc_sb = consts.tile([3, BPC * N], dt, tag="m2xc")
            nc.sync.dma_start(out=m2xc_sb[:], in_=m2xc_h[:])
            nv_sb = consts.tile([1, BPC * N], dt, tag="nv")
            nc.sync.dma_start(out=nv_sb[:], in_=nv_h[:])
            xp1_sb = consts.tile([128, BPC * 8], dt, tag="xp1")
            nc.sync.dma_start(out=xp1_sb[:], in_=xp1_h[:])
            omt_sb = consts.tile([128, 128 * (64 // S)], dt, tag="omt")
            nc.sync.dma_start(out=omt_sb[:], in_=omt_h[:])
            # fp32r copy: outer-product matmuls run at 1 cycle/row (vs 4 for
            # fp32) when the moving dim is >=256; operands must be produced
            # rounded, so round the stationary weights once here (DD is
            # rounded by the Sqrt activation's output stage).
            omt_r = consts.tile([128, 128 * (64 // S)], dtr, tag="omtr")
            nc.vector.tensor_copy(omt_r[:], omt_sb[:])
            phc_sb = consts.tile([128, 1], dt, tag="phc")
            nc.sync.dma_start(out=phc_sb[:], in_=phc_h[:])
            cmt_sb = consts.tile([128, 2 * S], dt, tag="cmt")
            nc.sync.dma_start(out=cmt_sb[:], in_=cmt_h[:])
            dmk_sb = consts.tile([128, 128], dt, tag="dmk")
            nc.sync.dma_start(out=dmk_sb[:], in_=dmk_h[:])
            ones_sb = consts.tile([128, 1], dt, tag="ones")
            nc.vector.memset(ones_sb[:], 1.0)
            eps_sb = consts.tile([128, 1], dt, tag="eps")
            nc.vector.memset(eps_sb[:], EPS)
            onesrow = consts.tile([1, N], dt, tag="onesrow")
            nc.vector.memset(onesrow[:], 1.0)
            trsb = consts.tile([1, BPC], dt, tag="trsb")

            # identity for PE transpose (= 1 - dmk)
            id_sb = consts.tile([128, 128], dt, tag="idm")
            nc.vector.tensor_scalar(id_sb[:], dmk_sb[:], -1.0, 1.0,
                                    Alu.mult, Alu.add)

            dd = []
            last_sqrt = None
            # ---- phase A: pairwise distances, all batches (batch ACT Sqrt).
            # F is symmetric, so per batch only compute the full (I0, all-j)
            # strip T0 (cols 0:256) plus the (I1, J1) block (cols 256:384);
            # the (I1, J0) quarter is recovered later by transposing (I0, J1).
            for b in range(BPC):
                dd_b = ddp.tile([128, 384], dtr, tag=f"dd{b}")
                dd.append(dd_b)
                o = N * b
                # T0: i in I0, j in 0:256
                ps = d2psum.tile([128, N], dt, tag="d2")
                nc.tensor.matmul(
                    ps[:], m2xc_sb[:, o:o + 128],
                    xc_sb[:, o:o + N], start=True, stop=False)
                nc.tensor.matmul(
                    ps[:], nv_sb[:, o:o + 128],
                    onesrow[:], start=False, stop=False)
                nc.tensor.matmul(
                    ps[:], onesrow[:, 0:128], nv_sb[:, o:o + N],
                    start=False, stop=True)
                d2sb = d2sbp.tile([128, N], dt, tag="d2sb")
                nc.vector.tensor_scalar(d2sb[:], ps[:], 0.0, None, Alu.max)
                nc.scalar.activation(dd_b[:, 0:N], d2sb[:],
                                     Act.Sqrt, bias=eps_sb[:])
                # T1': i in I1, j in J1 only
                ps2 = d2psum.tile([128, 128], dt, tag="d2")
                nc.tensor.matmul(
                    ps2[:], m2xc_sb[:, o + 128:o + N],
                    xc_sb[:, o + 128:o + N], start=True, stop=False)
                nc.tensor.matmul(
                    ps2[:], nv_sb[:, o + 128:o + N],
                    onesrow[:, 0:128], start=False, stop=False)
                nc.tensor.matmul(
                    ps2[:], onesrow[:, 0:128], nv_sb[:, o + 128:o + N],
                    start=False, stop=True)
                d2sb2 = d2sbp.tile([128, 128], dt, tag="d2sb")
                nc.vector.tensor_scalar(d2sb2[:], ps2[:], 0.0, None, Alu.max)
                last_sqrt = nc.scalar.activation(
                    dd_b[:, N:N + 128], d2sb2[:],
                    Act.Sqrt, bias=eps_sb[:])

            # ---- phases B/C per batch
            for b in range(BPC):
                stg = stgp.tile([128, 768], dt, tag="stg")
                pfill = None
                gpb = 64 // S              # groups per 64-row band (16)
                for zb in range(NG // 2):          # 2 groups per z-block
                    # each group's matmul output must stay inside one PSUM
                    # bank (512 f32), so groups sit at 512-col offsets and
                    # the Tanh reads them with a strided 3-D AP
                    zt = zpsum.tile([128, 1024], dt, tag="z")
                    for gi in range(2):
                        q = 2 * zb + gi
                        R, g = q // gpb, q % gpb
                        nc.tensor.matmul(
                            zt[:, 512 * gi:512 * gi + 384],
                            omt_r[64 * R:64 * R + 64, 128 * g:128 * g + 128],
                            dd[b][64 * R:64 * R + 64, :],
                            start=True, stop=True)
                    bas = basp.tile([128, 768], dt, tag="bas")
                    zt3 = zt[:].rearrange("p (g x) -> p g x", g=2)[:, :, 0:384]
                    bas3 = bas[:].rearrange("p (g x) -> p g x", g=2)
                    tanh_ins = nc.scalar.activation(bas3, zt3, Act.Tanh,
                                                    bias=phc_sb[:])
                    if zb == 0:
                        # keep all Sqrt activations ahead of the first Tanh of
                        # each batch so ACT switches table sets exactly once
                        add_dep_helper(tanh_ins.ins, last_sqrt.ins,
                                       reason="batch ACT table sets")
                    for gi in range(2):
                        q = 2 * zb + gi
                        if q % gpb == 0:
                            pfill = ppsum.tile([128, 384], dt, tag="p")
                        for c in range(3):
                            uo = 6 * S * (q % gpb) + 2 * S * c
                            nc.tensor.matmul(
                                pfill[:, uo:uo + 2 * S],
                                bas[:, 384 * gi + 128 * c:384 * gi + 128 * c + 128],
                                cmt_sb[:], start=True, stop=True)
                        if (q + 1) % gpb == 0:
                            half = q // gpb
                            nc.vector.tensor_copy(
                                stg[:, 384 * half:384 * half + 384], pfill[:])

                # de-interleave staging -> FT/GT tiles [j_in_tile, i]
                # c=0 -> FT0[:,0:128] (i in I0, j in J0)
                # c=1 -> FT1[:,0:128] (i in I0, j in J1)
                # c=2 -> FT1[:,128:256] (i in I1, j in J1)
                ft = [ftp.tile([128, 2 * N], dt, tag=f"ft{jt}", name=f"ft{jt}")
                      for jt in range(2)]
                gt = [ftp.tile([128, 2 * N], dt, tag=f"gt{jt}", name=f"gt{jt}")
                      for jt in range(2)]
                stg4 = stg[:].rearrange("p (f q w) -> p f q w", f=2, q=gpb)
                for c, (tile_, ih) in enumerate(((ft[0], 0), (ft[1], 0),
                                                 (ft[1], 1))):
                    dstf = tile_[:, 128 * ih:128 * ih + 128].rearrange(
                        "p (f q s) -> p f q s", f=2, q=gpb)
                    gtile = (gt[0], gt[1], gt[1])[c]
                    dstg = gtile[:, 128 * ih:128 * ih + 128].rearrange(
                        "p (f q s) -> p f q s", f=2, q=gpb)
                    nc.vector.tensor_copy(
                        dstf, stg4[:, :, :, 2 * S * c:2 * S * c + S])
                    nc.vector.tensor_copy(
                        dstg, stg4[:, :, :, 2 * S * c + S:2 * S * c + 2 * S])

                # recover (I1, J0) blocks: FT0[:,128:256] = FT1[:,0:128]^T
                sm = smpsum.tile([128, 136], dt, tag="sm")
                nc.tensor.transpose(sm[:, 8:136], ft[1][:, 0:128], id_sb[:])
                nc.vector.tensor_copy(ft[0][:, 128:256], sm[:, 8:136])
                nc.tensor.transpose(sm[:, 8:136], gt[1][:, 0:128], id_sb[:])
                nc.vector.tensor_copy(gt[0][:, 128:256], sm[:, 8:136])

                # zero the diagonal blocks
                nc.vector.tensor_tensor(ft[0][:, 0:128], ft[0][:, 0:128],
                                        dmk_sb[:], Alu.mult)
                nc.vector.tensor_tensor(ft[1][:, 128:256], ft[1][:, 128:256],
                                        dmk_sb[:], Alu.mult)
                nc.vector.tensor_tensor(gt[0][:, 0:128], gt[0][:, 0:128],
                                        dmk_sb[:], Alu.mult)
                nc.vector.tensor_tensor(gt[1][:, 128:256], gt[1][:, 128:256],
                                        dmk_sb[:], Alu.mult)

                # reductions: V4 = [F@x | S], per i-chunk; trace = sum g
                for ic in range(2):
                    for jt in range(2):
                        nc.tensor.matmul(
                            sm[:, 0:4], ft[jt][:, 128 * ic:128 * ic + 128],
                            xp1_sb[:, 8 * b + 4 * jt:8 * b + 4 * jt + 4],
                            start=(jt == 0), stop=(jt == 1))
                    vt = outp.tile([128, 4], dt, tag="vt")
                    nc.vector.tensor_copy(vt[:], sm[:, 0:4])
                    nc.sync.dma_start(
                        out=outv_h[b, 128 * ic:128 * ic + 128, :], in_=vt[:])
                for k, (jt, ic) in enumerate(
                        [(0, 0), (0, 1), (1, 0), (1, 1)]):
                    nc.tensor.matmul(
                        sm[:, 4:5], gt[jt][:, 128 * ic:128 * ic + 128],
                        ones_sb[:], start=(k == 0), stop=(k == 3))
                gcol = outp.tile([128, 1], dt, tag="gcol")
                nc.vector.tensor_copy(gcol[:], sm[:, 4:5])
                nc.tensor.matmul(sm[0:1, 5:6], gcol[:], ones_sb[:],
                                 start=True, stop=True)
                nc.vector.tensor_copy(trsb[:, b:b + 1], sm[0:1, 5:6])

            nc.sync.dma_start(out=outt_h[:], in_=trsb[:])

    nc.compile()
    return nc


def _get_nc():
    global _NC
    if _NC is None:
        _NC = _build_nc()
    return _NC


# -------------------------------------------------------------------- kernel
def kernel(t, x, W1, b1, W2, b2, W3, b3):
    from concourse.bass_utils import run_bass_kernel_spmd

    t = np.asarray(t); x = np.asarray(x)
    xr = np.asarray(x, np.float32).reshape(B, N, D)

    # global off-diagonal d range (float64 host scan)
    dmin, dmax = np.inf, 0.0
    for bb in range(B):
        xb = xr[bb].astype(np.float64)
        g2 = xb @ xb.T
        nvec = np.diag(g2).copy()
        d2 = nvec[:, None] + nvec[None, :] - 2.0 * g2
        np.fill_diagonal(d2, np.inf)
        dmin = min(dmin, float(d2.min()))
        np.fill_diagonal(d2, 0.0)
        dmax = max(dmax, float(d2.max()))
    dmin = np.sqrt(max(dmin, 0.0) + EPS)
    dmax = np.sqrt(dmax + EPS)

    sigma, beta, cf, cg = _fit(t, W1, b1, W2, b2, W3, b3, dmin, dmax, N_FEAT)

    S = S_STREAMS
    # omt: two identical 64-row bands (matmul base partition must be 0/64),
    # each holding 64//S variants side by side; variant g maps DD rows
    # S*g..S*g+S-1 of a 64-row slice to the 128 stacked (stream, feature)
    # outputs.
    gpb = 64 // S
    omt = np.zeros((128, 128 * gpb), np.float32)
    for band in range(2):
        for g in range(gpb):
            for s in range(S):
                omt[64 * band + S * g + s,
                    128 * g + N_FEAT * s:128 * g + N_FEAT * (s + 1)] = sigma
    phc = np.zeros((128, 1), np.float32)
    cmt = np.zeros((128, 2 * S), np.float32)
    for s in range(S):
        sl = slice(N_FEAT * s, N_FEAT * (s + 1))
        phc[sl, 0] = beta
        cmt[sl, s] = cf
        cmt[sl, S + s] = cg
    dmk = (1.0 - np.eye(128, dtype=np.float32))

    in_maps = []
    for c in range(NCORES):
        xs = xr[c * BPC:(c + 1) * BPC]                      # [BPC, N, 3]
        # xc: [3, BPC*N] — coordinate-major, batches along the free dim
        xc = np.ascontiguousarray(
            xs.transpose(2, 0, 1).reshape(3, BPC * N))
        nv = (xs * xs).sum(-1).astype(np.float32).reshape(1, BPC * N)
        xp1 = np.zeros((128, BPC * 8), np.float32)
        for bb in range(BPC):
            for jt in range(2):
                xp1[:, 8 * bb + 4 * jt:8 * bb + 4 * jt + 3] = \
                    xs[bb, 128 * jt:128 * jt + 128, :]
                xp1[:, 8 * bb + 4 * jt + 3] = 1.0
        in_maps.append(dict(
            xc=xc, m2xc=(-2.0 * xc).astype(np.float32), nv=nv, xp1=xp1,
            omt=omt, phc=phc, cmt=cmt, dmk=dmk))

    nc = _get_nc()
    res = run_bass_kernel_spmd(nc, in_maps, core_ids=list(range(NCORES)),
                               trace=_TRACE)
    global _LAST_RES
    _LAST_RES = res

    v = np.zeros((B, N, D), np.float32)
    trace = np.zeros((B, 1), np.float32)
    for c in range(NCORES):
        outv = res.results[c]["outv"]                       # [BPC, N, 4]
        outt = res.results[c]["outt"]                       # [1, BPC]
        xs = xr[c * BPC:(c + 1) * BPC]
        FX = outv[..., 0:3]
        Ssum = outv[..., 3]
        v[c * BPC:(c + 1) * BPC] = \
            (xs * Ssum[..., None] - FX) / np.float32(N - 1)
        trace[c * BPC:(c + 1) * BPC, 0] = outt[0] / np.float32(N)
    y = v - v.mean(axis=1, keepdims=True)
    return y.reshape(x.shape).astype(np.float32), trace


# revision 46
# speedup vs baseline: 1.0013x; 1.0013x over previous
"""Trainium2 kernel for nn_DistanceField.

Math: for each batch b, pairwise distances d_ij over N=256 particles feed a
small MLP f(d) (scalar), and the output field is
    v_i = (1/(N-1)) sum_{j!=i} (x_i - x_j) f(d_ij)
        = (x_i * S_i - (F @ x)_i) / (N-1),   S_i = sum_j F_ij,  F_ij = f(d_ij)
    y   = v - mean_i(v)
    trace_b = (1/N) * sum_{i,j!=i} g(d_ij),  g = 3 f + (d - EPS/d) f'(d)

f and g are smooth 1-D functions of d only (t and the MLP weights are shared
by every pair), so the device evaluates them via a tanh-feature fit
    f(d) ~= sum_m cf_m tanh(sigma_m d + beta_m)   (same basis for g)
fitted host-side in float64 against the exact MLP at kernel() time (ACT Tanh
is hardware-exact for any argument; Sin is only valid on [-pi, pi]). The
device computes, per batch: pairwise d via a Gram matmul + Sqrt, the tanh
basis via a tiny PE outer-product (fp32r, 1 cycle/row) + one ACT Tanh pass,
f/g via PE matmuls with the coefficient matrix, then the F-matrix reductions
(row sums, F @ x, total g) via PE matmuls. F is symmetric, so only the
(I0, all-j) strip and the (I1, J1) block of each 256x256 pair block are
evaluated; the (I1, J0) quarter is recovered with one PE transpose.
Sharding: data-parallel over batch, 4 batches per core on 8 cores.
Everything stays on-chip; no HBM intermediates.
"""
import numpy as np

B, N, D, H = 32, 256, 3, 64
EPS = 1e-6
NCORES = 8
BPC = B // NCORES          # batches per core
S_STREAMS = 8              # pair streams per outer-product group
N_FEAT = 128 // S_STREAMS  # tanh features per stream

_NC = None                 # cached Bass module
_TRACE = False             # set True to capture an NTFF trace (test harness)
_LAST_RES = None           # BassKernelResults of the last run


# ------------------------------------------------------------------ host math
def _mlp_fg(d, t, W1, b1, W2, b2, W3, b3):
    """Exact float64 f(d), g(d) of the reference MLP."""
    d = np.asarray(d, np.float64)
    W1 = np.asarray(W1, np.float64); b1 = np.asarray(b1, np.float64)
    W2 = np.asarray(W2, np.float64); b2 = np.asarray(b2, np.float64)
    W3 = np.asarray(W3, np.float64); b3 = np.asarray(b3, np.float64)
    t = np.float64(np.asarray(t).reshape(()))
    c = t * W1[1] + b1
    h1 = np.tanh(d[:, None] * W1[0][None, :] + c)
    h2 = np.tanh(h1 @ W2 + b2)
    f = (h2 @ W3)[:, 0] + b3[0]
    u1 = (1.0 - h1 * h1) * W1[0][None, :]
    u2 = (1.0 - h2 * h2) * (u1 @ W2)
    fp = (u2 @ W3)[:, 0]
    g = D * f + (d - EPS / d) * fp
    return f, g


def _fit(t, W1, b1, W2, b2, W3, b3, dmin, dmax, n_feat, sig_mult=0.5):
    """Fit f and g on [dmin,dmax] in a tanh-sigmoid basis
        basis_m(d) = tanh(sigma_m * d + beta_m)
    (feature 0 is the DC term tanh(37)=1). ACT Tanh is exact for any
    argument, unlike Sin which is only valid on [-pi,pi]. A short
    Levenberg-Marquardt pass (variable projection: coefficients re-solved in
    closed form per step) tunes the per-feature scales and centers, which
    buys ~40x accuracy over a fixed uniform layout at 16 features.
    Returns sigma, beta, cf, cg as float32 arrays."""
    pad = 0.05 * (dmax - dmin)
    a = max(dmin - pad, 1e-4)
    b = dmax + pad
    G = 4096
    center, L = 0.5 * (a + b), (b - a)
    u = np.cos(np.pi * (np.arange(G) + 0.5) / G)
    grid = np.concatenate([center + 0.5 * L * u, np.linspace(a, b, G)])
    fg, gg = _mlp_fg(grid, t, W1, b1, W2, b2, W3, b3)
    wg = np.abs(fg).max() / max(np.abs(gg).max(), 1e-30)
    # ridge strong enough to keep coefficients O(0.1): large canceling
    # coefficient pairs amplify the ~2^-13 fp32r rounding of the sigma*d
    # products on device
    lam = 3e-7 * len(grid)
    eye = np.eye(n_feat)

    def solve(theta):
        sig = theta[:n_feat - 1]
        tc = theta[n_feat - 1:]
        sigma = np.concatenate([[0.0], sig])
        beta = np.concatenate([[37.0], -sig * tc])
        A = np.tanh(np.outer(grid, sigma) + beta[None, :])
        ATA = A.T @ A + lam * eye
        cf = np.linalg.solve(ATA, A.T @ fg)
        cg = np.linalg.solve(ATA, A.T @ gg)
        r = np.concatenate([A @ cf - fg, wg * (A @ cg - gg)])
        return r, sigma, beta, cf, cg

    tk0 = np.linspace(a, b, n_feat - 1)
    theta = np.concatenate([np.full(n_feat - 1, sig_mult / (tk0[1] - tk0[0])),
                            tk0])
    best = solve(theta)
    best_theta, best_sse = theta, float((best[0] ** 2).sum())
    try:
        mu = 1e-3
        for _ in range(20):
            r0, *_ = solve(theta)
            J = np.zeros((len(r0), len(theta)))
            for j in range(len(theta)):
                dth = theta.copy()
                h = 1e-5 * max(1.0, abs(theta[j]))
                dth[j] += h
                J[:, j] = (solve(dth)[0] - r0) / h
            JTJ = J.T @ J
            JTr = J.T @ r0
            for _ in range(8):
                step = np.linalg.solve(
                    JTJ + mu * np.diag(np.diag(JTJ) + 1e-12), JTr)
                thn = theta - step
                rn, *_ = solve(thn)
                sse = float((rn ** 2).sum())
                if sse < float((r0 ** 2).sum()):
                    theta = thn
                    mu = max(mu / 3, 1e-8)
                    if sse < best_sse:
                        best_theta, best_sse = thn, sse
                    break
                mu *= 10
            else:
                break
    except np.linalg.LinAlgError:
        pass  # fall back to the best (possibly initial uniform) layout
    _, sigma, beta, cf, cg = solve(best_theta)
    return (sigma.astype(np.float32), beta.astype(np.float32),
            cf.astype(np.float32), cg.astype(np.float32))


# --------------------------------------------------------------- bass builder
def _build_nc():
    import concourse.mybir as mybir
    from concourse import bacc
    from concourse.tile import TileContext

    dt = mybir.dt.float32
    dtr = mybir.dt.float32r
    Alu = mybir.AluOpType
    Act = mybir.ActivationFunctionType
    from concourse.tile import add_dep_helper
    S = S_STREAMS
    NG = 128 // S              # groups per batch (16)
    nc = bacc.Bacc("TRN2", target_bir_lowering=False, debug=False)

    xc_h = nc.declare_dram_parameter("xc", [3, BPC * N], dt, isOutput=False)
    m2xc_h = nc.declare_dram_parameter("m2xc", [3, BPC * N], dt, isOutput=False)
    nv_h = nc.declare_dram_parameter("nv", [1, BPC * N], dt, isOutput=False)
    xp1_h = nc.declare_dram_parameter("xp1", [128, BPC * 8], dt, isOutput=False)
    omt_h = nc.declare_dram_parameter("omt", [128, 128 * (64 // S)], dt,
                                      isOutput=False)
    phc_h = nc.declare_dram_parameter("phc", [128, 1], dt, isOutput=False)
    cmt_h = nc.declare_dram_parameter("cmt", [128, 2 * S], dt, isOutput=False)
    dmk_h = nc.declare_dram_parameter("dmk", [128, 128], dt, isOutput=False)
    outv_h = nc.declare_dram_parameter("outv", [BPC, N, 4], dt, isOutput=True)
    outt_h = nc.declare_dram_parameter("outt", [1, BPC], dt, isOutput=True)

    with TileContext(nc) as tc:
        from contextlib import ExitStack
        with ExitStack() as ctx:
            consts = ctx.enter_context(tc.tile_pool(name="consts", bufs=1))
            ddp = ctx.enter_context(tc.tile_pool(name="dd", bufs=1))
            d2sbp = ctx.enter_context(tc.tile_pool(name="d2sb", bufs=3))
            basp = ctx.enter_context(tc.tile_pool(name="bas", bufs=4))
            stgp = ctx.enter_context(tc.tile_pool(name="stg", bufs=3))
            ftp = ctx.enter_context(tc.tile_pool(name="ft", bufs=3))
            outp = ctx.enter_context(tc.tile_pool(name="outs", bufs=2))
            zpsum = ctx.enter_context(tc.tile_pool(name="zp", bufs=2, space="PSUM"))
            ppsum = ctx.enter_context(tc.tile_pool(name="pp", bufs=1, space="PSUM"))
            d2psum = ctx.enter_context(tc.tile_pool(name="d2p", bufs=1, space="PSUM"))
            smpsum = ctx.enter_context(tc.tile_pool(name="smp", bufs=2, space="PSUM"))

            # ---- load constants / inputs
            xc_sb = consts.tile([3, BPC * N], dt, tag="xc")
            nc.sync.dma_start(out=xc_sb[:], in_=xc_h[:])
            m2xc_sb = consts.tile([3, BPC * N], dt, tag="m2xc")
            nc.sync.dma_start(out=m2xc_sb[:], in_=m2xc_h[:])
            nv_sb = consts.tile([1, BPC * N], dt, tag="nv")
            nc.sync.dma_start(out=nv_sb[:], in_=nv_h[:])
            xp1_sb = consts.tile([128, BPC * 8], dt, tag="xp1")
            nc.sync.dma_start(out=xp1_sb[:], in_=xp1_h[:])
            omt_sb = consts.tile([128, 128 * (64 // S)], dt, tag="omt")
            nc.sync.dma_start(out=omt_sb[:], in_=omt_h[:])
            # fp32r copy: outer-product matmuls run at 1 cycle/row (vs 4 for
            # fp32) when the moving dim is >=256; operands must be produced
            # rounded, so round the stationary weights once here (DD is
            # rounded by the Sqrt activation's output stage).
            omt_r = consts.tile([128, 128 * (64 // S)], dtr, tag="omtr")
            nc.vector.tensor_copy(omt_r[:], omt_sb[:])
            phc_sb = consts.tile([128, 1], dt, tag="phc")
            nc.sync.dma_start(out=phc_sb[:], in_=phc_h[:])
            cmt_sb = consts.tile([128, 2 * S], dt, tag="cmt")
            nc.sync.dma_start(out=cmt_sb[:], in_=cmt_h[:])
            dmk_sb = consts.tile([128, 128], dt, tag="dmk")
            nc.sync.dma_start(out=dmk_sb[:], in_=dmk_h[:])
            ones_sb = consts.tile([128, 1], dt, tag="ones")
            nc.vector.memset(ones_sb[:], 1.0)
            eps_sb = consts.tile([128, 1], dt, tag="eps")
            nc.vector.memset(eps_sb[:], EPS)
            onesrow = consts.tile([1, N], dt, tag="onesrow")
            nc.vector.memset(onesrow[:], 1.0)
            trsb = consts.tile([1, BPC], dt, tag="trsb")

            # identity for PE transpose (= 1 - dmk)
            id_sb = consts.tile([128, 128], dt, tag="idm")
            nc.vector.tensor_scalar(id_sb[:], dmk_sb[:], -1.0, 1.0,
                                    Alu.mult, Alu.add)

            dd = []
            last_sqrt = None
            # ---- phase A: pairwise distances, all batches (batch ACT Sqrt).
            # F is symmetric, so per batch only compute the full (I0, all-j)
            # strip T0 (cols 0:256) plus the (I1, J1) block (cols 256:384);
            # the (I1, J0) quarter is recovered later by transposing (I0, J1).
            for b in range(BPC):
                dd_b = ddp.tile([128, 384], dtr, tag=f"dd{b}")
                dd.append(dd_b)
                o = N * b
                # T0: i in I0, j in 0:256
                ps = d2psum.tile([128, N], dt, tag="d2")
                nc.tensor.matmul(
                    ps[:], m2xc_sb[:, o:o + 128],
                    xc_sb[:, o:o + N], start=True, stop=False)
                nc.tensor.matmul(
                    ps[:], nv_sb[:, o:o + 128],
                    onesrow[:], start=False, stop=False)
                nc.tensor.matmul(
                    ps[:], onesrow[:, 0:128], nv_sb[:, o:o + N],
                    start=False, stop=True)
                d2sb = d2sbp.tile([128, N], dt, tag="d2sb")
                nc.vector.tensor_scalar(d2sb[:], ps[:], 0.0, None, Alu.max)
                nc.scalar.activation(dd_b[:, 0:N], d2sb[:],
                                     Act.Sqrt, bias=eps_sb[:])
                # T1': i in I1, j in J1 only
                ps2 = d2psum.tile([128, 128], dt, tag="d2")
                nc.tensor.matmul(
                    ps2[:], m2xc_sb[:, o + 128:o + N],
                    xc_sb[:, o + 128:o + N], start=True, stop=False)
                nc.tensor.matmul(
                    ps2[:], nv_sb[:, o + 128:o + N],
                    onesrow[:, 0:128], start=False, stop=False)
                nc.tensor.matmul(
                    ps2[:], onesrow[:, 0:128], nv_sb[:, o + 128:o + N],
                    start=False, stop=True)
                d2sb2 = d2sbp.tile([128, 128], dt, tag="d2sb")
                nc.vector.tensor_scalar(d2sb2[:], ps2[:], 0.0, None, Alu.max)
                last_sqrt = nc.scalar.activation(
                    dd_b[:, N:N + 128], d2sb2[:],
                    Act.Sqrt, bias=eps_sb[:])

            # ---- phases B/C per batch
            for b in range(BPC):
                stg = stgp.tile([128, 768], dt, tag="stg")
                pfill = None
                gpb = 64 // S              # groups per 64-row band (16)
                for zb in range(NG // 2):          # 2 groups per z-block
                    # each group's matmul output must stay inside one PSUM
                    # bank (512 f32), so groups sit at 512-col offsets and
                    # the Tanh reads them with a strided 3-D AP
                    zt = zpsum.tile([128, 1024], dt, tag="z")
                    for gi in range(2):
                        q = 2 * zb + gi
                        R, g = q // gpb, q % gpb
                        nc.tensor.matmul(
                            zt[:, 512 * gi:512 * gi + 384],
                            omt_r[64 * R:64 * R + 64, 128 * g:128 * g + 128],
                            dd[b][64 * R:64 * R + 64, :],
                            start=True, stop=True)
                    bas = basp.tile([128, 768], dt, tag="bas")
                    zt3 = zt[:].rearrange("p (g x) -> p g x", g=2)[:, :, 0:384]
                    bas3 = bas[:].rearrange("p (g x) -> p g x", g=2)
                    tanh_ins = nc.scalar.activation(bas3, zt3, Act.Tanh,
                                                    bias=phc_sb[:])
                    if zb == 0:
                        # keep all Sqrt activations ahead of the first Tanh of
                        # each batch so ACT switches table sets exactly once
                        add_dep_helper(tanh_ins.ins, last_sqrt.ins,
                                       reason="batch ACT table sets")
                    for gi in range(2):
                        q = 2 * zb + gi
                        if q % gpb == 0:
                            pfill = ppsum.tile([128, 384], dt, tag="p")
                        for c in range(3):
                            uo = 6 * S * (q % gpb) + 2 * S * c
                            nc.tensor.matmul(
                                pfill[:, uo:uo + 2 * S],
                                bas[:, 384 * gi + 128 * c:384 * gi + 128 * c + 128],
                                cmt_sb[:], start=True, stop=True)
                        if (q + 1) % gpb == 0:
                            half = q // gpb
                            nc.vector.tensor_copy(
                                stg[:, 384 * half:384 * half + 384], pfill[:])

                # de-interleave staging -> FT/GT tiles [j_in_tile, i]
                # c=0 -> FT0[:,0:128] (i in I0, j in J0)
                # c=1 -> FT1[:,0:128] (i in I0, j in J1)
                # c=2 -> FT1[:,128:256] (i in I1, j in J1)
                ft = [ftp.tile([128, 2 * N], dt, tag=f"ft{jt}", name=f"ft{jt}")
                      for jt in range(2)]
                gt = [ftp.tile([128, 2 * N], dt, tag=f"gt{jt}", name=f"gt{jt}")
                      for jt in range(2)]
                stg4 = stg[:].rearrange("p (f q w) -> p f q w", f=2, q=gpb)
                for c, (tile_, ih) in enumerate(((ft[0], 0), (ft[1], 0),
                                                 (ft[1], 1))):
                    dstf = tile_[:, 128 * ih:128 * ih + 128].rearrange(
                        "p (f q s) -> p f q s", f=2, q=gpb)
                    gtile = (gt[0], gt[1], gt[1])[c]
                    dstg = gtile[:, 128 * ih:128 * ih + 128].rearrange(
                        "p (f q s) -> p f q s", f=2, q=gpb)
                    nc.vector.tensor_copy(
                        dstf, stg4[:, :, :, 2 * S * c:2 * S * c + S])
                    nc.vector.tensor_copy(
                        dstg, stg4[:, :, :, 2 * S * c + S:2 * S * c + 2 * S])

                # recover (I1, J0) blocks: FT0[:,128:256] = FT1[:,0:128]^T
                sm = smpsum.tile([128, 136], dt, tag="sm")
                nc.tensor.transpose(sm[:, 8:136], ft[1][:, 0:128], id_sb[:])
                nc.vector.tensor_copy(ft[0][:, 128:256], sm[:, 8:136])
                nc.tensor.transpose(sm[:, 8:136], gt[1][:, 0:128], id_sb[:])
                nc.vector.tensor_copy(gt[0][:, 128:256], sm[:, 8:136])

                # zero the diagonal blocks
                nc.vector.tensor_tensor(ft[0][:, 0:128], ft[0][:, 0:128],
                                        dmk_sb[:], Alu.mult)
                nc.vector.tensor_tensor(ft[1][:, 128:256], ft[1][:, 128:256],
                                        dmk_sb[:], Alu.mult)
                nc.vector.tensor_tensor(gt[0][:, 0:128], gt[0][:, 0:128],
                                        dmk_sb[:], Alu.mult)
                nc.vector.tensor_tensor(gt[1][:, 128:256], gt[1][:, 128:256],
                                        dmk_sb[:], Alu.mult)

                # reductions: V4 = [F@x | S], per i-chunk; trace = sum g
                for ic in range(2):
                    for jt in range(2):
                        nc.tensor.matmul(
                            sm[:, 0:4], ft[jt][:, 128 * ic:128 * ic + 128],
                            xp1_sb[:, 8 * b + 4 * jt:8 * b + 4 * jt + 4],
                            start=(jt == 0), stop=(jt == 1))
                    vt = outp.tile([128, 4], dt, tag="vt")
                    nc.vector.tensor_copy(vt[:], sm[:, 0:4])
                    nc.sync.dma_start(
                        out=outv_h[b, 128 * ic:128 * ic + 128, :], in_=vt[:])
                for k, (jt, ic) in enumerate(
                        [(0, 0), (0, 1), (1, 0), (1, 1)]):
                    nc.tensor.matmul(
                        sm[:, 4:5], gt[jt][:, 128 * ic:128 * ic + 128],
                        ones_sb[:], start=(k == 0), stop=(k == 3))
                gcol = outp.tile([128, 1], dt, tag="gcol")
                nc.vector.tensor_copy(gcol[:], sm[:, 4:5])
                nc.tensor.matmul(sm[0:1, 5:6], gcol[:], ones_sb[:],
                                 start=True, stop=True)
                nc.vector.tensor_copy(trsb[:, b:b + 1], sm[0:1, 5:6])

            nc.sync.dma_start(out=outt_h[:], in_=trsb[:])

    nc.compile()
    return nc


def _get_nc():
    global _NC
    if _NC is None:
        _NC = _build_nc()
    return _NC


# -------------------------------------------------------------------- kernel
def kernel(t, x, W1, b1, W2, b2, W3, b3):
    from concourse.bass_utils import run_bass_kernel_spmd

    t = np.asarray(t); x = np.asarray(x)
    xr = np.asarray(x, np.float32).reshape(B, N, D)

    # global off-diagonal d range (float64 host scan)
    dmin, dmax = np.inf, 0.0
    for bb in range(B):
        xb = xr[bb].astype(np.float64)
        g2 = xb @ xb.T
        nvec = np.diag(g2).copy()
        d2 = nvec[:, None] + nvec[None, :] - 2.0 * g2
        np.fill_diagonal(d2, np.inf)
        dmin = min(dmin, float(d2.min()))
        np.fill_diagonal(d2, 0.0)
        dmax = max(dmax, float(d2.max()))
    dmin = np.sqrt(max(dmin, 0.0) + EPS)
    dmax = np.sqrt(dmax + EPS)

    sigma, beta, cf, cg = _fit(t, W1, b1, W2, b2, W3, b3, dmin, dmax, N_FEAT)

    S = S_STREAMS
    # omt: two identical 64-row bands (matmul base partition must be 0/64),
    # each holding 64//S variants side by side; variant g maps DD rows
    # S*g..S*g+S-1 of a 64-row slice to the 128 stacked (stream, feature)
    # outputs.
    gpb = 64 // S
    omt = np.zeros((128, 128 * gpb), np.float32)
    for band in range(2):
        for g in range(gpb):
            for s in range(S):
                omt[64 * band + S * g + s,
                    128 * g + N_FEAT * s:128 * g + N_FEAT * (s + 1)] = sigma
    phc = np.zeros((128, 1), np.float32)
    cmt = np.zeros((128, 2 * S), np.float32)
    for s in range(S):
        sl = slice(N_FEAT * s, N_FEAT * (s + 1))
        phc[sl, 0] = beta
        cmt[sl, s] = cf
        cmt[sl, S + s] = cg
    dmk = (1.0 - np.eye(128, dtype=np.float32))

    in_maps = []
    for c in range(NCORES):
        xs = xr[c * BPC:(c + 1) * BPC]                      # [BPC, N, 3]
        # xc: [3, BPC*N] — coordinate-major, batches along the free dim
        xc = np.ascontiguousarray(
            xs.transpose(2, 0, 1).reshape(3, BPC * N))
        nv = (xs * xs).sum(-1).astype(np.float32).reshape(1, BPC * N)
        xp1 = np.zeros((128, BPC * 8), np.float32)
        for bb in range(BPC):
            for jt in range(2):
                xp1[:, 8 * bb + 4 * jt:8 * bb + 4 * jt + 3] = \
                    xs[bb, 128 * jt:128 * jt + 128, :]
                xp1[:, 8 * bb + 4 * jt + 3] = 1.0
        in_maps.append(dict(
            xc=xc, m2xc=(-2.0 * xc).astype(np.float32), nv=nv, xp1=xp1,
            omt=omt, phc=phc, cmt=cmt, dmk=dmk))

    nc = _get_nc()
    res = run_bass_kernel_spmd(nc, in_maps, core_ids=list(range(NCORES)),
                               trace=_TRACE)
    global _LAST_RES
    _LAST_RES = res

    v = np.zeros((B, N, D), np.float32)
    trace = np.zeros((B, 1), np.float32)
    for c in range(NCORES):
        outv = res.results[c]["outv"]                       # [BPC, N, 4]
        outt = res.results[c]["outt"]                       # [1, BPC]
        xs = xr[c * BPC:(c + 1) * BPC]
        FX = outv[..., 0:3]
        Ssum = outv[..., 3]
        v[c * BPC:(c + 1) * BPC] = \
            (xs * Ssum[..., None] - FX) / np.float32(N - 1)
        trace[c * BPC:(c + 1) * BPC, 0] = outt[0] / np.float32(N)
    y = v - v.mean(axis=1, keepdims=True)
    return y.reshape(x.shape).astype(np.float32), trace
